# revision 8
# baseline (speedup 1.0000x reference)
"""DCN CrossNetwork kernel for Trainium2 (8 NeuronCores, data-parallel).

Reference computation (B=16384, D=1024, L=4 layers):
    x0 = x
    for c in range(L):
        s = x_c @ w_c               # (B,) row-wise dot
        x_{c+1} = x0 * s[:,None] + b_c + x_c

Algebra: every iterate has the form  x_c = x0 * a_c + r_c  with a per-row
scalar a_c and a row-independent vector r_c = sum_{j<c} b_j.  Then
    s_c   = a_c * (x0 . w_c) + r_c . w_c
    a_{c+1} = a_c * (1 + U_c) + d_c,   U_c = x0 . w_c,  d_c = r_c . w_c
    out   = x0 * a_L + r_L
So the device kernel only needs U = x0 @ W^T (TensorE), a 4-step scan
(VectorE tensor_tensor_scan), and one fused (x0 * a + r4) op per tile
(scalar_tensor_tensor).  d_c / r_L are tiny host-precomputed constants
(O(L*D) work on the L x D parameters only).

Sharding: batch dim split across 8 cores (2048 rows each); weights/biases
replicated.  No collectives.
"""

import sys

for _p in ("/opt/trn_rl_repo",):
    if _p not in sys.path:
        sys.path.insert(0, _p)

import numpy as np

B, D, L = 16384, 1024, 4
N_CORES = 8
B_SHARD = B // N_CORES       # 2048 rows per core
P = 128                      # SBUF partitions
N_TILES = B_SHARD // P       # 16 row-tiles per core
N_CHUNKS = D // P            # 8 column-chunks of 128

_BUILT = None  # cached (nc) bass program


DEFAULT_CFG = dict(
    dma_batch=2,      # b-tiles per DMA transfer (1 or 2)
    copy_eng="act",   # xT PSUM->SBUF copy engines: "mix" (ACT+DVE), "act", "dve"
    x_bufs=6,
    xt_bufs=4,
    o_bufs=5,
    tp_bufs=4,
    up_bufs=2,
)


def build_bass(iters=1, **cfg_over):
    """Build the per-core Bass/Tile program (SPMD: same program, 8 cores).

    iters > 1 unrolls the whole body multiple times (same data) — used only
    for steady-state benchmarking via the loop-delta method.
    """
    import concourse.bass as bass
    import concourse.bacc as bacc
    import concourse.mybir as mybir
    import concourse.tile as tile

    cfg = {**DEFAULT_CFG, **cfg_over}
    f32 = mybir.dt.float32
    Alu = mybir.AluOpType
    Act = mybir.ActivationFunctionType

    # Bacc (not raw Bass): its compile() legalizes multi-sem-wait
    # instructions that this container's walrus codegen rejects.
    nc = bacc.Bacc("TRN2", debug=False)

    x_d = nc.dram_tensor("x", [B_SHARD, D], f32, kind="ExternalInput").ap()
    # wt[p, 4c+i] = W[i, 128c+p]  (W^T packed per 128-chunk)
    wt_d = nc.dram_tensor("wt", [P, L * N_CHUNKS], f32, kind="ExternalInput").ap()
    # r4 replicated across partitions
    r4_d = nc.dram_tensor("r4", [P, D], f32, kind="ExternalInput").ap()
    # d1[p, c] = d_c (replicated across partitions)
    d1_d = nc.dram_tensor("d1", [P, L], f32, kind="ExternalInput").ap()
    id_d = nc.dram_tensor("ident", [P, P], f32, kind="ExternalInput").ap()
    out_d = nc.dram_tensor("out", [B_SHARD, D], f32, kind="ExternalOutput").ap()

    NB = cfg["dma_batch"]
    assert N_TILES % NB == 0

    with tile.TileContext(nc) as tc:
        from contextlib import ExitStack

        with ExitStack() as ctx:
            cpool = ctx.enter_context(tc.tile_pool(name="consts", bufs=1))
            xpool = ctx.enter_context(tc.tile_pool(name="x", bufs=cfg["x_bufs"]))
            xtpool = ctx.enter_context(tc.tile_pool(name="xt", bufs=cfg["xt_bufs"]))
            opool = ctx.enter_context(tc.tile_pool(name="o", bufs=cfg["o_bufs"]))
            upool = ctx.enter_context(tc.tile_pool(name="u", bufs=3))
            apool = ctx.enter_context(tc.tile_pool(name="a", bufs=3))
            tpsum = ctx.enter_context(
                tc.tile_pool(name="tp", bufs=cfg["tp_bufs"], space=bass.MemorySpace.PSUM)
            )
            upsum = ctx.enter_context(
                tc.tile_pool(name="up", bufs=cfg["up_bufs"], space=bass.MemorySpace.PSUM)
            )

            wt_t = cpool.tile([P, L * N_CHUNKS], f32)
            nc.sync.dma_start(wt_t[:], wt_d[:])
            r4_t = cpool.tile([P, D], f32)
            nc.sync.dma_start(r4_t[:], r4_d[:])
            d1_t = cpool.tile([P, L], f32)
            nc.sync.dma_start(d1_t[:], d1_d[:])
            id_t = cpool.tile([P, P], f32)
            nc.sync.dma_start(id_t[:], id_d[:])

            for g in range((N_TILES // NB) * iters):
                g = g % (N_TILES // NB)
                r0 = g * NB * P
                # batched load: [NB*128, D] rows -> SBUF [128, NB, D]
                x_t = xpool.tile([P, NB, D], f32)
                nc.sync.dma_start(
                    x_t[:],
                    x_d[r0 : r0 + NB * P, :].rearrange("(nb p) d -> p nb d", p=P),
                )
                o_t = opool.tile([P, NB, D], f32)

                for s in range(NB):
                    x_s = x_t[:, s, :]
                    # --- transpose x tile chunk-wise via PE: xt[d, b] ---
                    xt_t = xtpool.tile([P, D], f32)
                    for h in range(2):  # two PSUM banks of 4 chunks each
                        tp = tpsum.tile([P, 512], f32)
                        for j in range(4):
                            c = 4 * h + j
                            nc.tensor.transpose(
                                tp[:, j * P : (j + 1) * P],
                                x_s[:, c * P : (c + 1) * P],
                                id_t[:],
                            )
                        # PSUM -> SBUF copy
                        ce = cfg["copy_eng"]
                        use_act = ce == "act" or (ce == "mix" and h == 0)
                        if use_act:
                            nc.scalar.copy(xt_t[:, h * 512 : (h + 1) * 512], tp[:])
                        else:
                            nc.vector.tensor_copy(
                                xt_t[:, h * 512 : (h + 1) * 512], tp[:]
                            )

                    # --- U[b, i] = sum_d x[b,d] W[i,d]: 8 accumulating matmuls ---
                    up = upsum.tile([P, L], f32)
                    for c in range(N_CHUNKS):
                        nc.tensor.matmul(
                            up[:],
                            xt_t[:, c * P : (c + 1) * P],   # lhsT [K=d, M=b]
                            wt_t[:, L * c : L * (c + 1)],   # rhs  [K=d, N=4]
                            start=(c == 0),
                            stop=(c == N_CHUNKS - 1),
                        )

                    # u1 = 1 + U  (fused into the PSUM->SBUF copy)
                    u1 = upool.tile([P, L], f32)
                    nc.scalar.activation(u1[:], up[:], Act.Copy, bias=1.0)

                    # scan: a_{c+1} = u1_c * a_c + d_c  -> a[:, 3] = a_4
                    a_t = apool.tile([P, L], f32)
                    nc.vector.tensor_tensor_scan(
                        a_t[:], u1[:], d1_t[:], initial=1.0,
                        op0=Alu.mult, op1=Alu.add,
                    )

                    # out = x0 * a4 + r4  (single fused op on DVE;
                    # GPSIMD lacks TensorScalarPtr on this ISA)
                    nc.vector.scalar_tensor_tensor(
                        o_t[:, s, :], x_s, a_t[:, L - 1 : L], r4_t[:],
                        op0=Alu.mult, op1=Alu.add,
                    )

                nc.sync.dma_start(
                    out_d[r0 : r0 + NB * P, :].rearrange("(nb p) d -> p nb d", p=P),
                    o_t[:],
                )

    nc.compile()
    return nc


def host_constants(weights, biases):
    """Pack W^T and precompute d_c / r4 (tiny O(L*D) host work)."""
    w = np.ascontiguousarray(np.asarray(weights, dtype=np.float32))
    b = np.ascontiguousarray(np.asarray(biases, dtype=np.float32))
    r = np.zeros(D, np.float32)
    d_vec = np.zeros(L, np.float32)
    for c in range(L):
        d_vec[c] = np.float32(r @ w[c])
        r = r + b[c]
    # wt[p, 4c+i] = W[i, 128c+p]
    wt = np.transpose(w.reshape(L, N_CHUNKS, P), (2, 1, 0)).reshape(P, N_CHUNKS * L)
    wt = np.ascontiguousarray(wt)
    r4_rep = np.ascontiguousarray(np.broadcast_to(r, (P, D)))
    d1_rep = np.ascontiguousarray(np.broadcast_to(d_vec, (P, L)))
    ident = np.eye(P, dtype=np.float32)
    return wt, r4_rep, d1_rep, ident


def _get_built():
    global _BUILT
    if _BUILT is None:
        _BUILT = build_bass()
    return _BUILT


def kernel(x, weights, biases, _trace=False):
    from concourse.bass_utils import run_bass_kernel_spmd

    x = np.ascontiguousarray(np.asarray(x, dtype=np.float32))
    assert x.shape == (B, D), x.shape
    wt, r4_rep, d1_rep, ident = host_constants(weights, biases)

    nc = _get_built()
    in_maps = []
    for c in range(N_CORES):
        in_maps.append(
            {
                "x": x[c * B_SHARD : (c + 1) * B_SHARD],
                "wt": wt,
                "r4": r4_rep,
                "d1": d1_rep,
                "ident": ident,
            }
        )
    res = run_bass_kernel_spmd(nc, in_maps, list(range(N_CORES)), trace=_trace)
    out = np.concatenate([res.results[c]["out"] for c in range(N_CORES)], axis=0)
    if _trace:
        kernel.last_results = res
    return out


# revision 15
# speedup vs baseline: 4.0182x; 4.0182x over previous
"""DCN CrossNetwork kernel for Trainium2 (8 NeuronCores, data-parallel).

Reference computation (B=16384, D=1024, L=4 layers):
    x0 = x
    for c in range(L):
        s = x_c @ w_c               # (B,) row-wise dot
        x_{c+1} = x0 * s[:,None] + b_c + x_c

Algebra: every iterate has the form  x_c = x0 * a_c + r_c  with a per-row
scalar a_c and a row-independent vector r_c = sum_{j<c} b_j.  Then
    s_c   = a_c * (x0 . w_c) + r_c . w_c
    a_{c+1} = a_c * (1 + U_c) + d_c,   U_c = x0 . w_c,  d_c = r_c . w_c
    out   = x0 * a_L + r_L
So the device kernel only needs U = x0 @ W^T (TensorE), a 4-step scan
(VectorE tensor_tensor_scan), and one fused (x0 * a + r4) op per tile
(scalar_tensor_tensor).  d_c / r_L are tiny host-precomputed constants
(O(L*D) work on the L x D parameters only).

Sharding: batch dim split across 8 cores (2048 rows each); weights/biases
replicated.  No collectives.
"""

import sys

for _p in ("/opt/trn_rl_repo",):
    if _p not in sys.path:
        sys.path.insert(0, _p)

import numpy as np

B, D, L = 16384, 1024, 4
N_CORES = 8
B_SHARD = B // N_CORES       # 2048 rows per core
P = 128                      # SBUF partitions
N_TILES = B_SHARD // P       # 16 row-tiles per core
N_CHUNKS = D // P            # 8 column-chunks of 128

_BUILT = None  # cached (nc) bass program


DEFAULT_CFG = dict(
    dma_batch=2,      # b-tiles per DMA transfer (1 or 2)
    copy_eng="act",   # xT PSUM->SBUF copy engines: "mix" (ACT+DVE), "act", "dve"
    x_bufs=6,
    xt_bufs=5,
    o_bufs=5,
    tp_bufs=3,
    up_bufs=2,
    sw_pipe=2,        # tiles of lag between transpose stage and U/final stage
    big_copy=True,    # one [128,1024] PSUM->SBUF copy per tile (tp = 2 banks)
    store_eng="sync",  # "sync" (SP queue) or "scalar" (ACT queue)
)


def build_bass(iters=1, **cfg_over):
    """Build the per-core Bass/Tile program (SPMD: same program, 8 cores).

    iters > 1 unrolls the whole body multiple times (same data) — used only
    for steady-state benchmarking via the loop-delta method.
    """
    import concourse.bass as bass
    import concourse.bacc as bacc
    import concourse.mybir as mybir
    import concourse.tile as tile

    cfg = {**DEFAULT_CFG, **cfg_over}
    f32 = mybir.dt.float32
    Alu = mybir.AluOpType
    Act = mybir.ActivationFunctionType

    # Bacc (not raw Bass): its compile() legalizes multi-sem-wait
    # instructions that this container's walrus codegen rejects.
    nc = bacc.Bacc("TRN2", debug=False)

    x_d = nc.dram_tensor("x", [B_SHARD, D], f32, kind="ExternalInput").ap()
    # wt[p, 4c+i] = W[i, 128c+p]  (W^T packed per 128-chunk)
    wt_d = nc.dram_tensor("wt", [P, L * N_CHUNKS], f32, kind="ExternalInput").ap()
    # r4 replicated across partitions
    r4_d = nc.dram_tensor("r4", [P, D], f32, kind="ExternalInput").ap()
    # d1[p, c] = d_c (replicated across partitions)
    d1_d = nc.dram_tensor("d1", [P, L], f32, kind="ExternalInput").ap()
    id_d = nc.dram_tensor("ident", [P, P], f32, kind="ExternalInput").ap()
    out_d = nc.dram_tensor("out", [B_SHARD, D], f32, kind="ExternalOutput").ap()

    NB = cfg["dma_batch"]
    assert N_TILES % NB == 0

    with tile.TileContext(nc) as tc:
        from contextlib import ExitStack

        with ExitStack() as ctx:
            cpool = ctx.enter_context(tc.tile_pool(name="consts", bufs=1))
            xpool = ctx.enter_context(tc.tile_pool(name="x", bufs=cfg["x_bufs"]))
            xtpool = ctx.enter_context(tc.tile_pool(name="xt", bufs=cfg["xt_bufs"]))
            opool = ctx.enter_context(tc.tile_pool(name="o", bufs=cfg["o_bufs"]))
            upool = ctx.enter_context(tc.tile_pool(name="u", bufs=3))
            apool = ctx.enter_context(tc.tile_pool(name="a", bufs=3))
            tpsum = ctx.enter_context(
                tc.tile_pool(name="tp", bufs=cfg["tp_bufs"], space=bass.MemorySpace.PSUM)
            )
            upsum = ctx.enter_context(
                tc.tile_pool(name="up", bufs=cfg["up_bufs"], space=bass.MemorySpace.PSUM)
            )

            wt_t = cpool.tile([P, L * N_CHUNKS], f32)
            nc.sync.dma_start(wt_t[:], wt_d[:])
            r4_t = cpool.tile([P, D], f32)
            nc.sync.dma_start(r4_t[:], r4_d[:])
            d1_t = cpool.tile([P, L], f32)
            nc.sync.dma_start(d1_t[:], d1_d[:])
            id_t = cpool.tile([P, P], f32)
            nc.sync.dma_start(id_t[:], id_d[:])

            # Software-pipelined emission: stage A (load/transpose/copy) runs
            # `sw_pipe` tiles ahead of stage B (U-matmul/scan/final/store) so
            # the PE never stalls on the PSUM->SBUF copy between its
            # transpose burst and U-matmul burst for the same tile.
            pend = []

            def emit_B(rec):
                xt_t, x_s, o_t, o_slice, grp = rec
                up = upsum.tile([P, L], f32)
                for c in range(N_CHUNKS):
                    nc.tensor.matmul(
                        up[:],
                        xt_t[:, c * P : (c + 1) * P],   # lhsT [K=d, M=b]
                        wt_t[:, L * c : L * (c + 1)],   # rhs  [K=d, N=4]
                        start=(c == 0),
                        stop=(c == N_CHUNKS - 1),
                    )
                # u1 = 1 + U  (fused into the PSUM->SBUF copy)
                u1 = upool.tile([P, L], f32)
                nc.scalar.activation(u1[:], up[:], Act.Copy, bias=1.0)
                # scan: a_{c+1} = u1_c * a_c + d_c  -> a[:, 3] = a_4
                a_t = apool.tile([P, L], f32)
                nc.vector.tensor_tensor_scan(
                    a_t[:], u1[:], d1_t[:], initial=1.0,
                    op0=Alu.mult, op1=Alu.add,
                )
                # out = x0 * a4 + r4  (single fused op on DVE;
                # GPSIMD lacks TensorScalarPtr on this ISA)
                nc.vector.scalar_tensor_tensor(
                    o_slice, x_s, a_t[:, L - 1 : L], r4_t[:],
                    op0=Alu.mult, op1=Alu.add,
                )
                grp["done"] += 1
                if grp["done"] == NB:
                    st = nc.scalar if cfg["store_eng"] == "scalar" else nc.sync
                    st.dma_start(grp["store_ap"], o_t[:])

            for g in range((N_TILES // NB) * iters):
                g = g % (N_TILES // NB)
                r0 = g * NB * P
                # batched load: [NB*128, D] rows -> SBUF [128, NB, D]
                x_t = xpool.tile([P, NB, D], f32)
                nc.sync.dma_start(
                    x_t[:],
                    x_d[r0 : r0 + NB * P, :].rearrange("(nb p) d -> p nb d", p=P),
                )
                o_t = opool.tile([P, NB, D], f32)
                grp = {
                    "done": 0,
                    "store_ap": out_d[r0 : r0 + NB * P, :].rearrange(
                        "(nb p) d -> p nb d", p=P
                    ),
                }

                for s in range(NB):
                    x_s = x_t[:, s, :]
                    # --- transpose x tile chunk-wise via PE: xt[d, b] ---
                    xt_t = xtpool.tile([P, D], f32)
                    ce = cfg["copy_eng"]
                    if cfg["big_copy"]:
                        tp = tpsum.tile([P, D], f32)  # spans 2 PSUM banks
                        for c in range(N_CHUNKS):
                            nc.tensor.transpose(
                                tp[:, c * P : (c + 1) * P],
                                x_s[:, c * P : (c + 1) * P],
                                id_t[:],
                            )
                        if ce == "dve":
                            nc.vector.tensor_copy(xt_t[:], tp[:])
                        else:
                            nc.scalar.copy(xt_t[:], tp[:])
                    else:
                        for h in range(2):  # two PSUM banks of 4 chunks each
                            tp = tpsum.tile([P, 512], f32)
                            for j in range(4):
                                c = 4 * h + j
                                nc.tensor.transpose(
                                    tp[:, j * P : (j + 1) * P],
                                    x_s[:, c * P : (c + 1) * P],
                                    id_t[:],
                                )
                            # PSUM -> SBUF copy
                            use_act = ce == "act" or (ce == "mix" and h == 0)
                            if use_act:
                                nc.scalar.copy(
                                    xt_t[:, h * 512 : (h + 1) * 512], tp[:]
                                )
                            else:
                                nc.vector.tensor_copy(
                                    xt_t[:, h * 512 : (h + 1) * 512], tp[:]
                                )
                    pend.append((xt_t, x_s, o_t, o_t[:, s, :], grp))
                    while len(pend) > cfg["sw_pipe"]:
                        emit_B(pend.pop(0))

            while pend:
                emit_B(pend.pop(0))

    nc.compile()
    return nc


def host_constants(weights, biases):
    """Pack W^T and precompute d_c / r4 (tiny O(L*D) host work)."""
    w = np.ascontiguousarray(np.asarray(weights, dtype=np.float32))
    b = np.ascontiguousarray(np.asarray(biases, dtype=np.float32))
    r = np.zeros(D, np.float32)
    d_vec = np.zeros(L, np.float32)
    for c in range(L):
        d_vec[c] = np.float32(r @ w[c])
        r = r + b[c]
    # wt[p, 4c+i] = W[i, 128c+p]
    wt = np.transpose(w.reshape(L, N_CHUNKS, P), (2, 1, 0)).reshape(P, N_CHUNKS * L)
    wt = np.ascontiguousarray(wt)
    r4_rep = np.ascontiguousarray(np.broadcast_to(r, (P, D)))
    d1_rep = np.ascontiguousarray(np.broadcast_to(d_vec, (P, L)))
    ident = np.eye(P, dtype=np.float32)
    return wt, r4_rep, d1_rep, ident


def _get_built():
    global _BUILT
    if _BUILT is None:
        _BUILT = build_bass()
    return _BUILT


def kernel(x, weights, biases, _trace=False):
    from concourse.bass_utils import run_bass_kernel_spmd

    x = np.ascontiguousarray(np.asarray(x, dtype=np.float32))
    assert x.shape == (B, D), x.shape
    wt, r4_rep, d1_rep, ident = host_constants(weights, biases)

    nc = _get_built()
    in_maps = []
    for c in range(N_CORES):
        in_maps.append(
            {
                "x": x[c * B_SHARD : (c + 1) * B_SHARD],
                "wt": wt,
                "r4": r4_rep,
                "d1": d1_rep,
                "ident": ident,
            }
        )
    res = run_bass_kernel_spmd(nc, in_maps, list(range(N_CORES)), trace=_trace)
    out = np.concatenate([res.results[c]["out"] for c in range(N_CORES)], axis=0)
    if _trace:
        kernel.last_results = res
    return out
